# revision 15
# baseline (speedup 1.0000x reference)
"""Causal squeeze-excite 1d on 8 TRN2 NeuronCores.

Reference computation (per batch b):
    y = causal_ema(x)                      # y[t] = (1-a) y[t-1] + a x[t], y[0] = x[0]
    h = relu(w1 @ y[:, t] + b1)            # (32,)  per time step
    g = sigmoid(w2 @ h + b2)               # (512,) per time step
    out[:, t] = x[:, t] * g

Sharding: data-parallel over batch. Core i gets x[2i:2i+2]; the tiny MLP
weights are replicated.

Key algebraic identity: the EMA is linear with channel-independent
coefficients, so it commutes with the channel projection:
    w1 @ ema(x) == ema(w1 @ x).
The kernel projects first (p = (a*w1) @ x on the TensorEngine, contracting
C=512) and scans p — a [32, T] sequence — instead of the [512, T] input.

The kernel is HBM-bandwidth-bound (~47 us of DMA at ~360 GB/s with fp16
I/O), so everything else is shaped to hide behind the DMA stream:

* All x/out HBM traffic is fp16: the host rounds x to fp16 (2^-11
  relative error, far inside the tolerance) and widens the fp16 output
  back. Halves DMA traffic and doubles DVE gate-multiply throughput
  (2x_1p packed mode).

* BATCH PAIRING: the core's two batch streams are stacked on the
  partition axis. mm1 writes batch 0's projection to PSUM partitions
  0-31 and batch 1's to 32-63 (sequential accumulation groups in one
  bank — interleaving them would trip the bank-wide accumulate-bit
  clear), so ONE tensor_tensor_scan [64, 512] and ONE relu advance both
  batches: DVE scan cost and ACT relu cost are halved (engine time
  scales with free size only). mm2 contracts each batch's half via
  matmul partition offsets (stationary w2 replicated at partitions
  32-63, HW-verified), and writes both batches into one [128, 1024]
  PSUM tile so each sigmoid also serves both batches at full N=1024
  efficiency.

* SOFTWARE PIPELINING: 8 units (time chunks of Tc=512 covering both
  batches) emitted with a 2/3-step skew —

      step s: load(s)      SP->HWDGE, one 512 KB DMA per batch
              front(s-2)   PE mm1, DVE scan-init + EMA scan, ACT relu
              back(s-3)    PE mm2, ACT sigmoid, DVE gate mul,
                           GPSIMD/SWDGE stores

  so each engine's in-order queue only sees work whose inputs landed
  steps earlier; the per-unit serial chain doesn't set the period.

* The scan init (u_0 = cw * p_0) runs on the DVE, and a dummy sigmoid
  pins the ACT table (sigmoid_and_others also contains relu) during the
  fill, so ACT never reloads tables mid-stream.
"""

import numpy as np
from contextlib import ExitStack

import concourse.bass as bass
import concourse.bacc as bacc
import concourse.tile as tile
import concourse.mybir as mybir
from concourse.bass_utils import run_bass_kernel_spmd

F32 = mybir.dt.float32
F16 = mybir.dt.float16

N_CORES = 8
B, C, T = 16, 512, 4096
CSQ = 32          # squeeze dim
P = 128           # SBUF partitions


def build_nc(B_loc, cw, C_=C, T_=T, Tc=512):
    """Build the per-core Bass program. Shapes are compile-time constants."""
    assert B_loc == 2, "pairing assumes two batches per core"
    d = 1.0 - 1.0 / cw
    NCB = C_ // P      # channel blocks
    NU = T_ // Tc      # pipeline units (time chunks, both batches each)
    CS2 = 2 * CSQ      # paired squeeze rows

    nc = bacc.Bacc(trn_type="TRN2")
    x = nc.declare_dram_parameter("x", [B_loc, C_, T_], F16, isOutput=False)
    w1sT = nc.declare_dram_parameter("w1sT", [C_, CSQ], F16, isOutput=False)
    b1r = nc.declare_dram_parameter("b1r", [CS2, 1], F32, isOutput=False)
    w2r = nc.declare_dram_parameter("w2r", [CS2, C_], F16, isOutput=False)
    b2 = nc.declare_dram_parameter("b2", [P, NCB], F32, isOutput=False)
    out = nc.declare_dram_parameter("out", [B_loc, C_, T_], F16, isOutput=True)

    with ExitStack() as ctx:
        tc = ctx.enter_context(tile.TileContext(nc))
        const = ctx.enter_context(tc.tile_pool(name="const", bufs=1))
        xpool = ctx.enter_context(tc.tile_pool(name="xp", bufs=10))
        opool = ctx.enter_context(tc.tile_pool(name="op", bufs=6))
        upool = ctx.enter_context(tc.tile_pool(name="up", bufs=4))
        hpool = ctx.enter_context(tc.tile_pool(name="hp", bufs=4))
        gpool = ctx.enter_context(tc.tile_pool(name="gp", bufs=8))
        ipool = ctx.enter_context(tc.tile_pool(name="ip", bufs=1))
        php = ctx.enter_context(tc.tile_pool(name="php", bufs=2, space="PSUM"))
        pgp = ctx.enter_context(tc.tile_pool(name="pgp", bufs=3, space="PSUM"))

        # ACT table warm-up during the DMA fill.
        warm = const.tile([CSQ, 1], F32, tag="warm")
        nc.vector.memset(warm[:], 0.0)
        nc.scalar.activation(warm[:], warm[:],
                             mybir.ActivationFunctionType.Sigmoid)

        dconst = const.tile([CS2, Tc], F32, tag="dconst")
        nc.vector.memset(dconst[:], d)
        # Constants ride the ACT HWDGE queue so the SP queue's first x loads
        # aren't stuck behind const dispatches.
        w1_t = []
        for cb in range(NCB):
            wt = const.tile([P, CSQ], F16, tag=f"w1_{cb}")
            nc.scalar.dma_start(wt[:], w1sT[cb * P:(cb + 1) * P, :])
            w1_t.append(wt)
        b1_t = const.tile([CS2, 1], F32, tag="b1")
        nc.scalar.dma_start(b1_t[:], b1r[:])
        w2_t = const.tile([CS2, C_], F16, tag="w2")
        nc.scalar.dma_start(w2_t[:], w2r[:])
        b2_t = const.tile([P, NCB], F32, tag="b2")
        nc.scalar.dma_start(b2_t[:], b2[:])

        # DRAM views with channel blocks split out: [B, P, NCB, T].
        xv = x.rearrange("b (cb p) t -> b p cb t", p=P)
        ov = out.rearrange("b (cb p) t -> b p cb t", p=P)

        xts, hts = {}, {}
        carry = [None]

        def emit_load(u):
            pair = []
            for b in range(B_loc):
                xt = xpool.tile([P, NCB * Tc], F16, tag=f"x{b}")
                nc.sync.dma_start(
                    xt[:], xv[b, :, :, u * Tc:(u + 1) * Tc])
                pair.append(xt)
            xts[u] = pair

        def emit_front(u):
            # p = (a*w1) @ x for both batches into one PSUM bank: batch b
            # occupies partitions 32b..32b+32. Groups must stay sequential
            # (a group's first matmul clears the whole bank's accum bits).
            ph = php.tile([CS2, Tc], F32, tag="ph")
            for b in range(B_loc):
                xt = xts[u][b]
                for cb in range(NCB):
                    nc.tensor.matmul(
                        ph[CSQ * b:CSQ * (b + 1), :], w1_t[cb][:],
                        xt[:, cb * Tc:(cb + 1) * Tc],
                        start=(cb == 0), stop=(cb == NCB - 1))
            # One EMA scan for both batches: u_t = d*u_{t-1} + p_t.
            ut = upool.tile([CS2, Tc], F32, tag="u")
            if carry[0] is None:
                it = ipool.tile([CS2, 1], F32, tag="i")
                nc.vector.tensor_scalar_mul(it[:], ph[:, 0:1], float(cw))
                init_ap = it[:]
            else:
                init_ap = carry[0][:, Tc - 1:Tc]
            nc.vector.tensor_tensor_scan(
                ut[:], dconst[:], ph[:], init_ap,
                mybir.AluOpType.mult, mybir.AluOpType.add)
            carry[0] = ut
            ht = hpool.tile([CS2, Tc], F16, tag="h")
            # Fused relu on the DVE: h = max(u + b1, 0). Keeps ACT (the
            # back-half pacer) sigmoid-only and the scan->relu chain on one
            # engine. (A GPSIMD version measured 8.7us/op — Q7 software path
            # is ~12x the cost model — so it lives here, not there.)
            nc.vector.tensor_scalar(
                ht[:], ut[:], b1_t[:], 0.0,
                mybir.AluOpType.add, mybir.AluOpType.max)
            hts[u] = ht

        def emit_back(u):
            ht = hts[u]
            ots = [opool.tile([P, NCB * Tc], F16, tag=f"o{b}", name=f"ot{b}")
                   for b in range(B_loc)]
            for cb in range(NCB):
                # Both batches' gates share one 2-bank PSUM tile and one
                # sigmoid: batch b in columns b*Tc..(b+1)*Tc.
                pg = pgp.tile([P, B_loc * Tc], F32, tag="pg")
                for b in range(B_loc):
                    nc.tensor.matmul(
                        pg[:, b * Tc:(b + 1) * Tc],
                        w2_t[CSQ * b:CSQ * (b + 1), cb * P:(cb + 1) * P],
                        ht[CSQ * b:CSQ * (b + 1), :], start=True, stop=True)
                gt = gpool.tile([P, B_loc * Tc], F16, tag="g")
                nc.scalar.activation(
                    gt[:], pg[:], mybir.ActivationFunctionType.Sigmoid,
                    bias=b2_t[:, cb:cb + 1])
                sl = slice(cb * Tc, (cb + 1) * Tc)
                for b in range(B_loc):
                    nc.vector.tensor_mul(
                        ots[b][:, sl], xts[u][b][:, sl],
                        gt[:, b * Tc:(b + 1) * Tc])
            for b in range(B_loc):
                nc.gpsimd.dma_start(
                    ov[b, :, :, u * Tc:(u + 1) * Tc], ots[b][:])

        for step in range(NU + 3):
            if step < NU:
                emit_load(step)
            if 0 <= step - 2 < NU:
                emit_front(step - 2)
            if 0 <= step - 3 < NU:
                emit_back(step - 3)
    nc.compile()
    return nc


def make_in_maps(x, w1, b1, w2, b2, cw, n_cores=N_CORES):
    """Host-side shard + weight prep. Returns per-core input maps."""
    a = 1.0 / cw
    w1sT = np.ascontiguousarray((w1.astype(np.float32) * a).T).astype(np.float16)
    b1c = np.ascontiguousarray(b1.reshape(-1, 1), dtype=np.float32)
    b1r = np.concatenate([b1c, b1c], axis=0)                         # [64, 1]
    w2T = np.ascontiguousarray(w2.T).astype(np.float16)              # [CSQ, C]
    w2r = np.concatenate([w2T, w2T], axis=0)                         # [64, C]
    ncb = w2.shape[0] // P
    b2c = np.ascontiguousarray(b2.reshape(ncb, P).T, dtype=np.float32)  # [P, NCB]
    b_loc = x.shape[0] // n_cores
    x16 = x.astype(np.float16)
    return [
        {
            "x": np.ascontiguousarray(x16[i * b_loc:(i + 1) * b_loc]),
            "w1sT": w1sT, "b1r": b1r, "w2r": w2r, "b2": b2c,
        }
        for i in range(n_cores)
    ]


_NC_CACHE = {}


def kernel(x, w1, b1, w2, b2, context_window):
    cw = int(context_window)
    x = np.asarray(x)
    key = (cw, x.shape)
    if key not in _NC_CACHE:
        _NC_CACHE[key] = build_nc(x.shape[0] // N_CORES, cw)
    nc = _NC_CACHE[key]
    in_maps = make_in_maps(
        np.asarray(x), np.asarray(w1), np.asarray(b1),
        np.asarray(w2), np.asarray(b2), cw)
    res = run_bass_kernel_spmd(nc, in_maps, core_ids=list(range(N_CORES)))
    return np.concatenate(
        [r["out"] for r in res.results], axis=0).astype(np.float32)


# revision 16
# speedup vs baseline: 1.0999x; 1.0999x over previous
"""Causal squeeze-excite 1d on 8 TRN2 NeuronCores.

Reference computation (per batch b):
    y = causal_ema(x)                      # y[t] = (1-a) y[t-1] + a x[t], y[0] = x[0]
    h = relu(w1 @ y[:, t] + b1)            # (32,)  per time step
    g = sigmoid(w2 @ h + b2)               # (512,) per time step
    out[:, t] = x[:, t] * g

Sharding: data-parallel over batch. Core i gets x[2i:2i+2]; the tiny MLP
weights are replicated.

Key algebraic identity: the EMA is linear with channel-independent
coefficients, so it commutes with the channel projection:
    w1 @ ema(x) == ema(w1 @ x).
The kernel projects first (p = (a*w1) @ x on the TensorEngine, contracting
C=512) and scans p — a [32, T] sequence — instead of the [512, T] input.

The kernel is HBM-bandwidth-bound (~47 us of DMA at ~360 GB/s with fp16
I/O), so everything else is shaped to hide behind the DMA stream:

* All x/out HBM traffic is fp16: the host rounds x to fp16 (2^-11
  relative error, far inside the tolerance) and widens the fp16 output
  back. Halves DMA traffic and doubles DVE gate-multiply throughput
  (2x_1p packed mode).

* BATCH PAIRING: the core's two batch streams are stacked on the
  partition axis. mm1 writes batch 0's projection to PSUM partitions
  0-31 and batch 1's to 32-63 (sequential accumulation groups in one
  bank — interleaving them would trip the bank-wide accumulate-bit
  clear), so ONE tensor_tensor_scan [64, 512] and ONE relu advance both
  batches: DVE scan cost and ACT relu cost are halved (engine time
  scales with free size only). mm2 contracts each batch's half via
  matmul partition offsets (stationary w2 replicated at partitions
  32-63, HW-verified), and writes both batches into one [128, 1024]
  PSUM tile so each sigmoid also serves both batches at full N=1024
  efficiency.

* SOFTWARE PIPELINING: 8 units (time chunks of Tc=512 covering both
  batches) emitted with a 2/3-step skew —

      step s: load(s)      SP->HWDGE, one 512 KB DMA per batch
              front(s-2)   PE mm1, DVE scan-init + EMA scan, ACT relu
              back(s-3)    PE mm2, ACT sigmoid, DVE gate mul,
                           GPSIMD/SWDGE stores

  so each engine's in-order queue only sees work whose inputs landed
  steps earlier; the per-unit serial chain doesn't set the period.

* The scan init (u_0 = cw * p_0) runs on the DVE, and a dummy sigmoid
  pins the ACT table (sigmoid_and_others also contains relu) during the
  fill, so ACT never reloads tables mid-stream.
"""

import numpy as np
from contextlib import ExitStack

import concourse.bass as bass
import concourse.bacc as bacc
import concourse.tile as tile
import concourse.mybir as mybir
from concourse.bass_utils import run_bass_kernel_spmd

F32 = mybir.dt.float32
F16 = mybir.dt.float16

N_CORES = 8
B, C, T = 16, 512, 4096
CSQ = 32          # squeeze dim
P = 128           # SBUF partitions


def build_nc(B_loc, cw, C_=C, T_=T, Tc=512):
    """Build the per-core Bass program. Shapes are compile-time constants."""
    assert B_loc == 2, "pairing assumes two batches per core"
    d = 1.0 - 1.0 / cw
    NCB = C_ // P      # channel blocks
    NU = T_ // Tc      # pipeline units (time chunks, both batches each)
    CS2 = 2 * CSQ      # paired squeeze rows

    nc = bacc.Bacc(trn_type="TRN2")
    x = nc.declare_dram_parameter("x", [B_loc, C_, T_], F16, isOutput=False)
    w1sT = nc.declare_dram_parameter("w1sT", [C_, CSQ], F16, isOutput=False)
    b1r = nc.declare_dram_parameter("b1r", [CS2, 1], F32, isOutput=False)
    w2r = nc.declare_dram_parameter("w2r", [CS2, C_], F16, isOutput=False)
    b2 = nc.declare_dram_parameter("b2", [P, NCB], F32, isOutput=False)
    out = nc.declare_dram_parameter("out", [B_loc, C_, T_], F16, isOutput=True)

    with ExitStack() as ctx:
        tc = ctx.enter_context(tile.TileContext(nc))
        const = ctx.enter_context(tc.tile_pool(name="const", bufs=1))
        xpool = ctx.enter_context(tc.tile_pool(name="xp", bufs=10))
        opool = ctx.enter_context(tc.tile_pool(name="op", bufs=6))
        upool = ctx.enter_context(tc.tile_pool(name="up", bufs=4))
        hpool = ctx.enter_context(tc.tile_pool(name="hp", bufs=4))
        gpool = ctx.enter_context(tc.tile_pool(name="gp", bufs=8))
        ipool = ctx.enter_context(tc.tile_pool(name="ip", bufs=1))
        php = ctx.enter_context(tc.tile_pool(name="php", bufs=2, space="PSUM"))
        pgp = ctx.enter_context(tc.tile_pool(name="pgp", bufs=3, space="PSUM"))

        # ACT table warm-up during the DMA fill.
        warm = const.tile([CSQ, 1], F32, tag="warm")
        nc.vector.memset(warm[:], 0.0)
        nc.scalar.activation(warm[:], warm[:],
                             mybir.ActivationFunctionType.Sigmoid)

        dconst = const.tile([CS2, Tc], F32, tag="dconst")
        nc.vector.memset(dconst[:], d)
        # Constants ride the ACT HWDGE queue so the SP queue's first x loads
        # aren't stuck behind const dispatches.
        w1_t = []
        for cb in range(NCB):
            wt = const.tile([P, CSQ], F16, tag=f"w1_{cb}")
            nc.scalar.dma_start(wt[:], w1sT[cb * P:(cb + 1) * P, :])
            w1_t.append(wt)
        b1_t = const.tile([CS2, 1], F32, tag="b1")
        nc.scalar.dma_start(b1_t[:], b1r[:])
        w2_t = const.tile([CS2, C_], F16, tag="w2")
        nc.scalar.dma_start(w2_t[:], w2r[:])
        b2_t = const.tile([P, NCB], F32, tag="b2")
        nc.scalar.dma_start(b2_t[:], b2[:])

        # DRAM views with channel blocks split out: [B, P, NCB, T].
        xv = x.rearrange("b (cb p) t -> b p cb t", p=P)
        ov = out.rearrange("b (cb p) t -> b p cb t", p=P)

        xts, hts = {}, {}
        carry = [None]

        def emit_load(u):
            pair = []
            for b in range(B_loc):
                xt = xpool.tile([P, NCB * Tc], F16, tag=f"x{b}")
                nc.sync.dma_start(
                    xt[:], xv[b, :, :, u * Tc:(u + 1) * Tc])
                pair.append(xt)
            xts[u] = pair

        def emit_front(u):
            # p = (a*w1) @ x for both batches into one PSUM bank: batch b
            # occupies partitions 32b..32b+32. Groups must stay sequential
            # (a group's first matmul clears the whole bank's accum bits).
            ph = php.tile([CS2, Tc], F32, tag="ph")
            for b in range(B_loc):
                xt = xts[u][b]
                for cb in range(NCB):
                    nc.tensor.matmul(
                        ph[CSQ * b:CSQ * (b + 1), :], w1_t[cb][:],
                        xt[:, cb * Tc:(cb + 1) * Tc],
                        start=(cb == 0), stop=(cb == NCB - 1))
            # One EMA scan for both batches: u_t = d*u_{t-1} + p_t.
            ut = upool.tile([CS2, Tc], F32, tag="u")
            if carry[0] is None:
                it = ipool.tile([CS2, 1], F32, tag="i")
                nc.vector.tensor_scalar_mul(it[:], ph[:, 0:1], float(cw))
                init_ap = it[:]
            else:
                init_ap = carry[0][:, Tc - 1:Tc]
            nc.vector.tensor_tensor_scan(
                ut[:], dconst[:], ph[:], init_ap,
                mybir.AluOpType.mult, mybir.AluOpType.add)
            carry[0] = ut
            ht = hpool.tile([CS2, Tc], F16, tag="h")
            # relu stays on ACT with the +b1 bias fused. Measured
            # alternatives both lose: GPSIMD tensor_scalar is 8.7us/op (Q7
            # software path, ~12x the cost model), and a DVE tensor_scalar
            # relu lengthens the DVE front chain (scan -> relu serialize on
            # one engine) for a net +6us end-to-end.
            nc.scalar.activation(
                ht[:], ut[:], mybir.ActivationFunctionType.Relu, bias=b1_t[:])
            hts[u] = ht

        def emit_back(u):
            ht = hts[u]
            ots = [opool.tile([P, NCB * Tc], F16, tag=f"o{b}", name=f"ot{b}")
                   for b in range(B_loc)]
            for cb in range(NCB):
                # Both batches' gates share one 2-bank PSUM tile and one
                # sigmoid: batch b in columns b*Tc..(b+1)*Tc.
                pg = pgp.tile([P, B_loc * Tc], F32, tag="pg")
                for b in range(B_loc):
                    nc.tensor.matmul(
                        pg[:, b * Tc:(b + 1) * Tc],
                        w2_t[CSQ * b:CSQ * (b + 1), cb * P:(cb + 1) * P],
                        ht[CSQ * b:CSQ * (b + 1), :], start=True, stop=True)
                gt = gpool.tile([P, B_loc * Tc], F16, tag="g")
                nc.scalar.activation(
                    gt[:], pg[:], mybir.ActivationFunctionType.Sigmoid,
                    bias=b2_t[:, cb:cb + 1])
                sl = slice(cb * Tc, (cb + 1) * Tc)
                for b in range(B_loc):
                    nc.vector.tensor_mul(
                        ots[b][:, sl], xts[u][b][:, sl],
                        gt[:, b * Tc:(b + 1) * Tc])
            for b in range(B_loc):
                nc.gpsimd.dma_start(
                    ov[b, :, :, u * Tc:(u + 1) * Tc], ots[b][:])

        for step in range(NU + 3):
            if step < NU:
                emit_load(step)
            if 0 <= step - 2 < NU:
                emit_front(step - 2)
            if 0 <= step - 3 < NU:
                emit_back(step - 3)
    nc.compile()
    return nc


def make_in_maps(x, w1, b1, w2, b2, cw, n_cores=N_CORES):
    """Host-side shard + weight prep. Returns per-core input maps."""
    a = 1.0 / cw
    w1sT = np.ascontiguousarray((w1.astype(np.float32) * a).T).astype(np.float16)
    b1c = np.ascontiguousarray(b1.reshape(-1, 1), dtype=np.float32)
    b1r = np.concatenate([b1c, b1c], axis=0)                         # [64, 1]
    w2T = np.ascontiguousarray(w2.T).astype(np.float16)              # [CSQ, C]
    w2r = np.concatenate([w2T, w2T], axis=0)                         # [64, C]
    ncb = w2.shape[0] // P
    b2c = np.ascontiguousarray(b2.reshape(ncb, P).T, dtype=np.float32)  # [P, NCB]
    b_loc = x.shape[0] // n_cores
    x16 = x.astype(np.float16)
    return [
        {
            "x": np.ascontiguousarray(x16[i * b_loc:(i + 1) * b_loc]),
            "w1sT": w1sT, "b1r": b1r, "w2r": w2r, "b2": b2c,
        }
        for i in range(n_cores)
    ]


_NC_CACHE = {}


def kernel(x, w1, b1, w2, b2, context_window):
    cw = int(context_window)
    x = np.asarray(x)
    key = (cw, x.shape)
    if key not in _NC_CACHE:
        _NC_CACHE[key] = build_nc(x.shape[0] // N_CORES, cw)
    nc = _NC_CACHE[key]
    in_maps = make_in_maps(
        np.asarray(x), np.asarray(w1), np.asarray(b1),
        np.asarray(w2), np.asarray(b2), cw)
    res = run_bass_kernel_spmd(nc, in_maps, core_ids=list(range(N_CORES)))
    return np.concatenate(
        [r["out"] for r in res.results], axis=0).astype(np.float32)
